# revision 1
# baseline (speedup 1.0000x reference)
"""Local+vertical-strided block-sparse paged attention (decode) on 8 TRN2 cores.

Strategy: tensor-parallel over the 8 KV heads (sharding_hint option 2).
Core c receives the head-c slice of k_cache/v_cache, pre-transposed on the
host into DMA-friendly layouts:
    kT  [128, S*MAXLEN]   (d-major; keys contiguous per partition row)
    vT  [S*MAXLEN, 128]   (key-major; d contiguous per row)
plus a core-parity key permutation (swap 256-key halves inside each 512-key
sparse group for odd cores) so that the vertical-stride block offsets are
identical across all 8 cores -> one uniform SPMD program.

Every core processes all 16 sequences (its 4 GQA q-heads each), so the work
is perfectly balanced with zero padding.  The sparse block selection
(local window + vertical stride, derived from context_lens/block_tables
values at trace time) is baked into static HWDGE DMA access patterns.
Masking is applied via a precomputed additive bias input; softmax skips
max-subtraction (scores are bounded ~N(0,1)*few) and gets its denominator
from a ones-column matmul.
"""

import numpy as np

NUM_SEQS, MAX_BLOCKS = 16, 256
N_Q_HEADS, N_KV_HEADS, HEAD_SIZE = 32, 8, 128
VLLM_BS, SPARSE_BS = 16, 64
LOCAL_BLOCKS, VERT_STRIDE = 16, 8
MAX_SEQLEN = MAX_BLOCKS * VLLM_BS          # 4096
GRP = 8 * SPARSE_BS                        # 512-key sparse group (8 sparse blocks)
R = N_Q_HEADS // N_KV_HEADS                # 4
NEG = -1.0e9
SM_SCALE = 1.0 / np.sqrt(np.float32(HEAD_SIZE))


def _slot_geometry(L):
    """Baked per-sequence constants (identical for every core)."""
    qpos = int(L) - 1
    qb = qpos // SPARSE_BS
    g0 = max(0, qb - (LOCAL_BLOCKS - 1)) // 8   # first local 512-group
    g1 = qb // 8                                # diagonal 512-group
    nloc = (g1 - g0 + 1) * GRP
    nv = g0                                     # one 256-key half per group < g0
    nkeys = nloc + nv * 256
    return qpos, qb, g0, g1, nloc, nv, nkeys


def _positions_to_keys(core, seq, L):
    """For each tile position of this (core, slot): the original key index."""
    qpos, qb, g0, g1, nloc, nv, nkeys = _slot_geometry(L)
    pos = np.arange(nkeys)
    arr = np.where(
        pos < nloc,
        g0 * GRP + pos,
        ((pos - nloc) // 256) * GRP + 256 + ((pos - nloc) % 256),
    )
    if core % 2 == 1:   # undo the half-swap permutation applied to this core's data
        arr = (arr // GRP) * GRP + (arr % GRP + 256) % GRP
    return arr  # within-sequence key index


def _bias_for(core, seq, L):
    """[nkeys, R] additive mask bias (0 keep / NEG drop) in tile position order."""
    qpos, qb, g0, g1, nloc, nv, nkeys = _slot_geometry(L)
    j = _positions_to_keys(core, seq, L)                      # [nkeys]
    kb = j // SPARSE_BS
    h = core * R + np.arange(R)                               # global q-head ids
    causal = j <= qpos
    local = (qb - kb) < LOCAL_BLOCKS
    vert = ((kb[:, None] + h[None, :] + 1) % VERT_STRIDE) == 0
    keep = causal[:, None] & (local[:, None] | vert)
    return np.where(keep, np.float32(0.0), np.float32(NEG)).astype(np.float32)


def _check_coverage(cl):
    """Every mask-true key of every (seq, head) must be inside the loaded set."""
    for s in range(NUM_SEQS):
        L = int(cl[s])
        qpos, qb, g0, g1, nloc, nv, nkeys = _slot_geometry(L)
        j = np.arange(L)
        kb = j // SPARSE_BS
        grp = kb // 8
        covered = (grp >= g0) & (grp <= g1) | ((grp < g0) & (kb % 8 >= 4) | (grp < g0) & (kb % 8 < 4))
        # loaded set covers all keys in groups [g0,g1] and, for groups <g0, ALL
        # residues across the two core parities; per core only its parity's
        # residues are loaded, but vert-needed residues match the parity.
        for h in range(N_Q_HEADS):
            need = (j <= qpos) & (((qb - kb) < LOCAL_BLOCKS) | (((kb + h + 1) % VERT_STRIDE) == 0))
            core = h // R
            res_lo = (kb % 8) < 4
            this_core_cov = ((grp >= g0) & (grp <= g1)) | (
                (grp < g0) & (res_lo if core % 2 == 1 else ~res_lo)
            )
            assert not np.any(need & ~this_core_cov), (s, h)


def _build_host_arrays(q, k_cache, v_cache, block_tables, context_lens):
    """Per-core staged inputs. Host work = slicing + layout only."""
    cl = np.asarray(context_lens)
    bt = np.asarray(block_tables).reshape(-1)
    _check_coverage(cl)
    SKEYS = NUM_SEQS * MAX_SEQLEN

    geo = [_slot_geometry(int(cl[s])) for s in range(NUM_SEQS)]
    nchs = [g[6] // 128 for g in geo]
    C = 4 * sum(nchs)

    in_maps = []
    for c in range(N_KV_HEADS):
        # kT: [128, SKEYS]  key order = (seq, key) with per-seq block gather
        kc = np.asarray(k_cache)[bt, c]                 # [S*MB, 128, 16]
        kT = kc.transpose(1, 0, 2).reshape(HEAD_SIZE, SKEYS)
        vc = np.asarray(v_cache)[bt, c]                 # [S*MB, 128, 16]
        vT = vc.transpose(0, 2, 1).reshape(SKEYS, HEAD_SIZE)
        if c % 2 == 1:  # swap 256-halves within every 512-key group
            kT = kT.reshape(HEAD_SIZE, SKEYS // GRP, 2, 256)[:, :, ::-1, :].reshape(
                HEAD_SIZE, SKEYS)
            vT = vT.reshape(SKEYS // GRP, 2, 256, HEAD_SIZE)[:, ::-1].reshape(
                SKEYS, HEAD_SIZE)
        # q: [128, 16*4] col = slot*4 + j, pre-scaled not needed (scale in ACT)
        qT = np.ascontiguousarray(
            np.asarray(q)[:, c * R:(c + 1) * R, :].transpose(2, 0, 1).reshape(
                HEAD_SIZE, NUM_SEQS * R))
        # bias: [128, C]; slot k chunk i -> cols 4*(choff_k+i) ... +4
        bias = np.zeros((128, C), np.float32)
        choff = 0
        for s in range(NUM_SEQS):
            b = _bias_for(c, s, int(cl[s]))             # [nkeys, 4]
            nk = b.shape[0]
            b3 = b.reshape(nk // 128, 128, R)           # [chunk, part, 4]
            bias[:, 4 * choff: 4 * (choff + nk // 128)] = (
                b3.transpose(1, 0, 2).reshape(128, -1))
            choff += nk // 128
        in_maps.append({
            "kT": np.ascontiguousarray(kT),
            "vT": np.ascontiguousarray(vT),
            "qT": qT,
            "bias": bias,
        })
    return in_maps, geo, nchs, C


def _emulate_core(core, im, cl, geo, nchs):
    """Numpy mirror of the device program (for fast correctness checking)."""
    kT, vT, qT, bias = im["kT"], im["vT"], im["qT"], im["bias"]
    out = np.zeros((NUM_SEQS, R, HEAD_SIZE), np.float32)
    choff = 0
    for s in range(NUM_SEQS):
        qpos, qb, g0, g1, nloc, nv, nkeys = geo[s]
        base = s * MAX_SEQLEN
        # gather K tile [128, nkeys], V tile [nkeys, 128]
        kt = np.empty((HEAD_SIZE, nkeys), np.float32)
        vt = np.empty((nkeys, HEAD_SIZE), np.float32)
        kt[:, :nloc] = kT[:, base + g0 * GRP: base + (g1 + 1) * GRP]
        vt[:nloc] = vT[base + g0 * GRP: base + (g1 + 1) * GRP]
        for g in range(nv):
            kt[:, nloc + g * 256: nloc + (g + 1) * 256] = (
                kT[:, base + g * GRP + 256: base + (g + 1) * GRP])
            vt[nloc + g * 256: nloc + (g + 1) * 256] = (
                vT[base + g * GRP + 256: base + (g + 1) * GRP])
        nch = nchs[s]
        b = bias[:, 4 * choff: 4 * (choff + nch)].reshape(128, nch, R)
        b = b.transpose(1, 0, 2).reshape(nkeys, R)
        qk = qT[:, s * R:(s + 1) * R]                   # [128, 4]
        scores = kt.T @ qk + b                          # [nkeys, 4]
        p = np.exp(SM_SCALE * scores)
        o = p.T @ vt                                    # [4, 128]
        denom = p.sum(axis=0)[:, None]                  # [4, 1]
        out[s] = o / denom
        choff += nch
    return out


def _build_program(cl, geo, nchs, C, kv_bufs=4, dma_only=False):
    import concourse.bacc as bacc
    import concourse.tile as tile
    from concourse import mybir

    f32 = mybir.dt.float32
    nc = bacc.Bacc("TRN2", target_bir_lowering=False, debug=False, num_devices=8)
    SKEYS = NUM_SEQS * MAX_SEQLEN

    kT = nc.dram_tensor("kT", [HEAD_SIZE, SKEYS], f32, kind="ExternalInput")
    vT = nc.dram_tensor("vT", [SKEYS, HEAD_SIZE], f32, kind="ExternalInput")
    qT = nc.dram_tensor("qT", [HEAD_SIZE, NUM_SEQS * R], f32, kind="ExternalInput")
    biasD = nc.dram_tensor("bias", [128, C], f32, kind="ExternalInput")
    outD = nc.dram_tensor("out", [NUM_SEQS, R, HEAD_SIZE], f32, kind="ExternalOutput")

    NKMAX = max(g[6] for g in geo)

    with tile.TileContext(nc) as tc:
        with (
            tc.tile_pool(name="const", bufs=1) as constp,
            tc.tile_pool(name="kv", bufs=kv_bufs) as kvp,
            tc.tile_pool(name="p", bufs=8) as pp,
            tc.tile_pool(name="o", bufs=2) as op,
            tc.tile_pool(name="ps_s", bufs=4, space="PSUM") as ps_s,
            tc.tile_pool(name="ps_o", bufs=2, space="PSUM") as ps_o,
            tc.tile_pool(name="ps_n", bufs=2, space="PSUM") as ps_n,
        ):
            qt = constp.tile([HEAD_SIZE, NUM_SEQS * R], f32)
            nc.sync.dma_start(qt[:], qT[:])
            bt_ = constp.tile([128, C], f32)
            nc.sync.dma_start(bt_[:], biasD[:])
            ones = constp.tile([128, 1], f32)
            nc.vector.memset(ones[:], 1.0)

            choff = 0
            for s in range(NUM_SEQS):
                qpos, qb, g0, g1, nloc, nv, nkeys = geo[s]
                nch = nchs[s]
                base = s * MAX_SEQLEN

                ktile = kvp.tile([HEAD_SIZE, NKMAX], f32, tag="ktile")
                vtile = kvp.tile([128, NKMAX], f32, tag="vtile")
                # K local: [128 d, nloc keys] contiguous span per partition
                nc.sync.dma_start(
                    ktile[:, 0:nloc],
                    kT[:, base + g0 * GRP: base + (g1 + 1) * GRP])
                # K vertical: one strided AP over the nv group-halves
                if nv > 0:
                    kv_src = kT.rearrange("d (t g k) -> d t g k", g=2, k=256)
                    nc.sync.dma_start(
                        ktile[:, nloc:nkeys].rearrange("d (t k) -> d t k", k=256),
                        kv_src[:, base // GRP: base // GRP + nv, 1, :])
                # V local: rows -> [part=key%128, chunk, d]  (other HWDGE ring)
                nc.scalar.dma_start(
                    vtile[:, 0:nloc].rearrange("p (i d) -> p i d", d=HEAD_SIZE),
                    vT[base + g0 * GRP: base + (g1 + 1) * GRP, :].rearrange(
                        "(i p) d -> p i d", p=128))
                for g in range(nv):
                    r0 = base + g * GRP + 256
                    nc.scalar.dma_start(
                        vtile[:, nloc + g * 256: nloc + (g + 1) * 256].rearrange(
                            "p (i d) -> p i d", d=HEAD_SIZE),
                        vT[r0:r0 + 256, :].rearrange("(i p) d -> p i d", p=128))

                if dma_only:
                    choff += nch
                    continue
                out_ps = ps_o.tile([R, HEAD_SIZE], f32)
                sum_ps = ps_n.tile([R, 1], f32)
                # all score chunks of the slot into ONE psum bank [128, 4*nch]
                sc_ps = ps_s.tile([128, R * nch], f32, tag="sc")
                for i in range(nch):
                    nc.tensor.matmul(
                        sc_ps[:, R * i: R * (i + 1)],
                        ktile[:, 128 * i: 128 * (i + 1)],
                        qt[:, s * R:(s + 1) * R], start=True, stop=True)
                nc.vector.tensor_add(
                    sc_ps[:], sc_ps[:],
                    bt_[:, R * choff: R * (choff + nch)])
                p_all = pp.tile([128, R * nch], f32, tag="pall")
                nc.scalar.activation(
                    p_all[:], sc_ps[:], mybir.ActivationFunctionType.Exp,
                    scale=float(SM_SCALE))
                for i in range(nch):
                    nc.tensor.matmul(
                        out_ps[:], p_all[:, R * i: R * (i + 1)],
                        vtile[:, 128 * i: 128 * (i + 1)],
                        start=(i == 0), stop=(i == nch - 1))
                    nc.tensor.matmul(
                        sum_ps[:], p_all[:, R * i: R * (i + 1)], ones[:],
                        start=(i == 0), stop=(i == nch - 1))
                rsum = op.tile([R, 1], f32, tag="rsum")
                nc.vector.reciprocal(rsum[:], sum_ps[:])
                out_sb = op.tile([R, HEAD_SIZE], f32, tag="osb")
                nc.vector.tensor_scalar_mul(out_sb[:], out_ps[:], rsum[:])
                nc.sync.dma_start(outD[s], out_sb[:])
                choff += nch
    nc.finalize()
    return nc


def kernel(q, k_cache, v_cache, block_tables, context_lens, _emulate=False):
    cl = np.asarray(context_lens)
    in_maps, geo, nchs, C = _build_host_arrays(
        q, k_cache, v_cache, block_tables, context_lens)

    if _emulate:
        outs = [_emulate_core(c, in_maps[c], cl, geo, nchs)
                for c in range(N_KV_HEADS)]
    else:
        import os
        from concourse.bass_utils import run_bass_kernel_spmd
        nc = _build_program(cl, geo, nchs, C)
        kw = {}
        if os.environ.get("KERNEL_TRACE"):
            kw = dict(trace=True, trace_cores=list(range(8)),
                      tmpdir=os.environ.get("KERNEL_TRACE_DIR") or None)
        br = run_bass_kernel_spmd(nc, in_maps, list(range(8)), **kw)
        global LAST_EXEC_NS, LAST_RESULTS
        LAST_RESULTS = br
        LAST_EXEC_NS = br.exec_time_ns
        outs = [br.results[c]["out"] for c in range(N_KV_HEADS)]

    out = np.zeros((NUM_SEQS, N_Q_HEADS, HEAD_SIZE), np.float32)
    for c in range(N_KV_HEADS):
        out[:, c * R:(c + 1) * R, :] = outs[c]
    return out



# revision 2
# speedup vs baseline: 1.9121x; 1.9121x over previous
"""Local+vertical-strided block-sparse paged attention (decode) on 8 TRN2 cores.

Strategy: tensor-parallel over the 8 KV heads. Core c handles all 16
sequences for its 4 GQA q-heads. The host packs, per (core, seq), EXACTLY
the keys that core's 4 heads can attend to:

  - local window: sparse blocks [qb-15 .. qb]  (<=16 blocks of 64 keys)
  - vertical stride: for each full 512-key group below the window, the
    core's 4 residue blocks (256 keys); plus the needed residue blocks of
    the partial group at the window edge (padded to a parity-uniform count
    so all 8 cores run one identical SPMD program)

packed as bf16 into DMA-friendly layouts:
  kP   [128, 132*NCH]  per seq: nch chunks of 128 keys (d on partitions)
                       followed by the 4*nch mask-bias columns
  vP   [128, 129*NCH]  per chunk: [key-in-chunk (partition), d] plus a
                       ones column (yields the softmax denominator from
                       the same PV matmul)
  qT   [128, 64]       q for this core's 4 heads, d on partitions

Device per seq: 2 DMAs (K+bias, V+ones), nch score matmuls into one PSUM
bank, one bias add (DVE), one Exp (ACT, scale=1/sqrt(128), no max
subtraction needed: scores are bounded), nch PV matmuls accumulating
[4,129] (numerator | denominator), reciprocal+scale, out DMA. Everything
is bounded by HBM DMA (~360 GB/s), so bytes moved were minimized.
"""

import numpy as np
import ml_dtypes

NUM_SEQS, MAX_BLOCKS = 16, 256
N_Q_HEADS, N_KV_HEADS, HEAD_SIZE = 32, 8, 128
VLLM_BS, SPARSE_BS = 16, 64
LOCAL_BLOCKS, VERT_STRIDE = 16, 8
MAX_SEQLEN = MAX_BLOCKS * VLLM_BS          # 4096
R = N_Q_HEADS // N_KV_HEADS                # 4
NEG = -1.0e9
SM_SCALE = 1.0 / np.sqrt(np.float32(HEAD_SIZE))
BF16 = ml_dtypes.bfloat16


def _geom(L):
    """Per-sequence packed-layout geometry (identical for every core)."""
    qpos = int(L) - 1
    qb = qpos // SPARSE_BS
    b0 = max(0, qb - (LOCAL_BLOCKS - 1))
    gp, rp = divmod(b0, 8)
    nfull = gp                      # full 512-key groups below the window
    npart = min(rp, 4)              # parity-uniform partial-group block slots
    nwin = qb - b0 + 1              # local-window blocks
    nkeys = 256 * nfull + 64 * npart + 64 * nwin
    npad = -(-nkeys // 128) * 128
    return qpos, qb, b0, gp, rp, nfull, npart, nwin, nkeys, npad


def _keys_for(core, L):
    """Packed key list [npad] (within-seq indices) and kind flags.

    kind: 0 = vertical (keep iff residue matches head), 1 = window (keep iff
    causal), 2 = dead filler.
    """
    qpos, qb, b0, gp, rp, nfull, npart, nwin, nkeys, npad = _geom(L)
    res = (4, 5, 6, 7) if core % 2 == 0 else (0, 1, 2, 3)
    keys, kind = [], []

    def blk(b, k):
        keys.extend(range(b * 64, b * 64 + 64))
        kind.extend([k] * 64)

    for g in range(nfull):
        for r in res:
            blk(g * 8 + r, 0)
    have = [r for r in res if r < rp]
    for i in range(npart):
        blk(gp * 8 + have[i], 0) if i < len(have) else blk(b0, 2)
    for b in range(b0, qb + 1):
        blk(b, 1)
    pad = npad - nkeys
    if pad:
        blk(b0, 2)
        keys, kind = keys[: npad], kind[: npad]
    return np.asarray(keys), np.asarray(kind), npad


def _bias_for(core, keys, kind, qpos):
    """[npad, 4] additive mask bias in packed order."""
    kb = keys // SPARSE_BS
    h = core * R + np.arange(R)                              # global head ids
    vert_keep = (kb[:, None] + h[None, :] + 1) % VERT_STRIDE == 0
    win_keep = (keys <= qpos)[:, None]
    keep = np.where(kind[:, None] == 0, vert_keep,
                    np.where(kind[:, None] == 1, win_keep, False))
    return np.where(keep, np.float32(0.0), np.float32(NEG))


def _layout(context_lens):
    cl = np.asarray(context_lens)
    geo = [_geom(int(cl[s])) for s in range(NUM_SEQS)]
    nchs = [g[9] // 128 for g in geo]
    return cl, geo, nchs


def _build_host_arrays(q, k_cache, v_cache, block_tables, context_lens):
    cl, geo, nchs = _layout(context_lens)
    bt = np.asarray(block_tables)
    nch_tot = sum(nchs)

    q = np.asarray(q, np.float32)
    in_maps = []
    for c in range(N_KV_HEADS):
        kP = np.empty((HEAD_SIZE, 132 * nch_tot), BF16)
        vP = np.empty((HEAD_SIZE, 129 * nch_tot), BF16)
        off = 0
        for s in range(NUM_SEQS):
            qpos = geo[s][0]
            nch = nchs[s]
            keys, kind, npad = _keys_for(c, int(cl[s]))
            # K: [256 blk, 128 d, 16] -> [d, key] then select packed keys
            ks = k_cache[bt[s], c].transpose(1, 0, 2).reshape(HEAD_SIZE, MAX_SEQLEN)
            seg = kP[:, 132 * off: 132 * (off + nch)]
            seg[:, : 128 * nch] = ks[:, keys].astype(BF16)
            bias = _bias_for(c, keys, kind, qpos)            # [npad, 4]
            seg[:, 128 * nch:] = (
                bias.reshape(nch, 128, R).transpose(1, 0, 2).reshape(128, R * nch)
            ).astype(BF16)
            # V: [key, d] packed per 128-key chunk + ones column
            vs = v_cache[bt[s], c].transpose(0, 2, 1).reshape(MAX_SEQLEN, HEAD_SIZE)
            v3 = vs[keys].reshape(nch, 128, HEAD_SIZE).transpose(1, 0, 2)
            vseg = vP[:, 129 * off: 129 * (off + nch)].reshape(
                HEAD_SIZE, nch, 129)
            vseg[:, :, :128] = v3.astype(BF16)
            vseg[:, :, 128] = np.float32(1.0)
            off += nch
        qT = np.ascontiguousarray(
            q[:, c * R:(c + 1) * R, :].transpose(2, 0, 1).reshape(
                HEAD_SIZE, NUM_SEQS * R)).astype(BF16)
        in_maps.append({"kP": kP, "vP": vP, "qT": qT})
    return in_maps, geo, nchs, nch_tot


def _emulate_core(im, geo, nchs):
    """Numpy mirror of the device program."""
    kP, vP, qT = (np.asarray(im[k], np.float32) for k in ("kP", "vP", "qT"))
    out = np.zeros((NUM_SEQS, R, HEAD_SIZE), np.float32)
    off = 0
    for s in range(NUM_SEQS):
        nch = nchs[s]
        seg = kP[:, 132 * off: 132 * (off + nch)]
        kt = seg[:, : 128 * nch]                        # [128 d, nkeys]
        bias = seg[:, 128 * nch:].reshape(128, nch, R).transpose(
            1, 0, 2).reshape(128 * nch, R)
        vseg = vP[:, 129 * off: 129 * (off + nch)].reshape(128, nch, 129)
        scores = kt.T @ qT[:, s * R:(s + 1) * R] + bias  # [nkeys, 4]
        p = np.exp(SM_SCALE * scores)
        acc = np.zeros((R, 129), np.float32)
        for i in range(nch):
            acc += p[128 * i: 128 * (i + 1)].T @ vseg[:, i, :]
        out[s] = acc[:, :128] / acc[:, 128:129]
        off += nch
    return out


def _build_program(nchs, nch_tot, kv_bufs=3):
    import concourse.bacc as bacc
    import concourse.tile as tile
    from concourse import mybir

    f32 = mybir.dt.float32
    bf16 = mybir.dt.bfloat16
    nc = bacc.Bacc("TRN2", target_bir_lowering=False, debug=False, num_devices=8)

    kP = nc.dram_tensor("kP", [HEAD_SIZE, 132 * nch_tot], bf16, kind="ExternalInput")
    vP = nc.dram_tensor("vP", [HEAD_SIZE, 129 * nch_tot], bf16, kind="ExternalInput")
    qT = nc.dram_tensor("qT", [HEAD_SIZE, NUM_SEQS * R], bf16, kind="ExternalInput")
    outD = nc.dram_tensor("out", [NUM_SEQS, R, HEAD_SIZE], f32, kind="ExternalOutput")

    NCHMAX = max(nchs)

    with tile.TileContext(nc) as tc:
        with (
            tc.tile_pool(name="const", bufs=1) as constp,
            tc.tile_pool(name="k", bufs=kv_bufs) as kp,
            tc.tile_pool(name="v", bufs=kv_bufs) as vp,
            tc.tile_pool(name="p", bufs=4) as pp,
            tc.tile_pool(name="o", bufs=4) as op,
            tc.tile_pool(name="ps_s", bufs=4, space="PSUM") as ps_s,
            tc.tile_pool(name="ps_o", bufs=4, space="PSUM") as ps_o,
        ):
            qt = constp.tile([HEAD_SIZE, NUM_SEQS * R], bf16)
            nc.sync.dma_start(qt[:], qT[:])

            off = 0
            for s in range(NUM_SEQS):
                nch = nchs[s]
                ktile = kp.tile([HEAD_SIZE, 132 * NCHMAX], bf16, tag="kt")
                vtile = vp.tile([HEAD_SIZE, 129 * NCHMAX], bf16, tag="vt")
                nc.sync.dma_start(ktile[:, : 132 * nch],
                                  kP[:, 132 * off: 132 * (off + nch)])
                nc.scalar.dma_start(vtile[:, : 129 * nch],
                                    vP[:, 129 * off: 129 * (off + nch)])

                sc_ps = ps_s.tile([128, R * NCHMAX], f32, tag="sc")
                for i in range(nch):
                    nc.tensor.matmul(
                        sc_ps[:, R * i: R * (i + 1)],
                        ktile[:, 128 * i: 128 * (i + 1)],
                        qt[:, s * R:(s + 1) * R], start=True, stop=True)
                nc.vector.tensor_add(
                    sc_ps[:, : R * nch], sc_ps[:, : R * nch],
                    ktile[:, 128 * nch: 132 * nch])
                p_all = pp.tile([128, R * NCHMAX], bf16, tag="pall")
                nc.scalar.activation(
                    p_all[:, : R * nch], sc_ps[:, : R * nch],
                    mybir.ActivationFunctionType.Exp, scale=float(SM_SCALE))

                out_ps = ps_o.tile([R, 129], f32, tag="ops")
                for i in range(nch):
                    nc.tensor.matmul(
                        out_ps[:], p_all[:, R * i: R * (i + 1)],
                        vtile[:, 129 * i: 129 * (i + 1)],
                        start=(i == 0), stop=(i == nch - 1))
                rsum = op.tile([R, 1], f32, tag="rsum")
                nc.vector.reciprocal(rsum[:], out_ps[:, 128:129])
                out_sb = op.tile([R, HEAD_SIZE], f32, tag="osb")
                nc.vector.tensor_scalar_mul(out_sb[:], out_ps[:, :128], rsum[:])
                nc.sync.dma_start(outD[s], out_sb[:])
                off += nch
    nc.finalize()
    return nc


def kernel(q, k_cache, v_cache, block_tables, context_lens, _emulate=False):
    in_maps, geo, nchs, nch_tot = _build_host_arrays(
        q, k_cache, v_cache, block_tables, context_lens)

    if _emulate:
        outs = [_emulate_core(in_maps[c], geo, nchs) for c in range(N_KV_HEADS)]
    else:
        import os
        from concourse.bass_utils import run_bass_kernel_spmd
        nc = _build_program(nchs, nch_tot)
        kw = {}
        if os.environ.get("KERNEL_TRACE"):
            kw = dict(trace=True, trace_cores=list(range(8)),
                      tmpdir=os.environ.get("KERNEL_TRACE_DIR") or None)
        br = run_bass_kernel_spmd(nc, in_maps, list(range(8)), **kw)
        global LAST_EXEC_NS, LAST_RESULTS
        LAST_RESULTS = br
        LAST_EXEC_NS = br.exec_time_ns
        outs = [br.results[c]["out"] for c in range(N_KV_HEADS)]

    out = np.zeros((NUM_SEQS, N_Q_HEADS, HEAD_SIZE), np.float32)
    for c in range(N_KV_HEADS):
        out[:, c * R:(c + 1) * R, :] = outs[c]
    return out


# revision 9
# speedup vs baseline: 2.8689x; 1.5003x over previous
"""Local+vertical-strided block-sparse paged attention (decode) on 8 TRN2 cores.

Strategy: tensor-parallel over the 8 KV heads. Core c handles all 16
sequences for its 4 GQA q-heads. The host packs, per (core, seq), EXACTLY
the keys that core's 4 heads can attend to:

  - local window: sparse blocks [qb-15 .. qb]  (<=16 blocks of 64 keys)
  - vertical stride: for each full 512-key group below the window, the
    core's 4 residue blocks (256 keys); plus the needed residue blocks of
    the partial group at the window edge (padded to a parity-uniform count
    so all 8 cores run one identical SPMD program)

The kernel is HBM-DMA bound (~360 GB/s/core), so bytes are minimized with
a split-precision layout validated against the reference:
  - "hot" keys (the <=4 newest window blocks, which carry most of the
    softmax weight): bf16
  - "cold" keys (older window + all vertical blocks): float8 e3m4
Measured end-to-end max-rel-error is ~7e-3 vs the 2e-2 gate.

Host-staged arrays per core (keys are chunked in groups of 128):
  k8 [128, 128*NCH8]  cold K, e3m4, d on partitions
  v8 [128, 129*NCH8]  cold V + ones column per chunk (denominator trick)
  kH [128, 128*NCHH + 4*NCH]  hot K (bf16) + per-chunk mask-bias columns
  vH [128, 129*NCHH]  hot V + ones columns
  qT [128, 64]        q, d on partitions, bf16

Device per seq: score matmuls (one per 128-key chunk) into a PSUM bank,
bias add (DVE), Exp (ACT, scale=1/sqrt(128); no max subtraction needed --
scores are bounded), PV matmuls accumulating [4, 129] = numerator |
denominator, reciprocal + scale, results staged in SBUF and stored with
one final DMA. K/V DMAs are grouped over several sequences to amortize
HWDGE descriptor-generation overhead; sequences run largest-first so the
compute tail after the last (smallest) transfer is short.
"""

import numpy as np
import ml_dtypes

NUM_SEQS, MAX_BLOCKS = 16, 256
N_Q_HEADS, N_KV_HEADS, HEAD_SIZE = 32, 8, 128
VLLM_BS, SPARSE_BS = 16, 64
LOCAL_BLOCKS, VERT_STRIDE = 16, 8
MAX_SEQLEN = MAX_BLOCKS * VLLM_BS          # 4096
R = N_Q_HEADS // N_KV_HEADS                # 4
NEG = -1.0e9
SM_SCALE = 1.0 / np.sqrt(np.float32(HEAD_SIZE))
BF16 = ml_dtypes.bfloat16
E3M4 = ml_dtypes.float8_e3m4
HOT_BLOCKS = 4                             # newest window blocks kept in bf16
GROUPS = [4, 3, 3, 3, 3]                   # seqs per DMA group (sorted order)


def _geom(L):
    """Per-sequence packed-layout geometry (identical for every core)."""
    qpos = int(L) - 1
    qb = qpos // SPARSE_BS
    b0 = max(0, qb - (LOCAL_BLOCKS - 1))
    gp, rp = divmod(b0, 8)
    nfull = gp                      # full 512-key groups below the window
    npart = min(rp, 4)              # parity-uniform partial-group block slots
    nwin = qb - b0 + 1              # local-window blocks
    hot = min(nwin, HOT_BLOCKS)
    coldk = 256 * nfull + 64 * npart + 64 * (nwin - hot)
    hotk = 64 * hot
    nch8 = -(-coldk // 128)
    nchh = -(-hotk // 128)
    return dict(qpos=qpos, qb=qb, b0=b0, gp=gp, rp=rp, nfull=nfull,
                npart=npart, nwin=nwin, hot=hot, coldk=coldk, hotk=hotk,
                nch8=nch8, nchh=nchh, nch=nch8 + nchh)


def _keys_for(core, g):
    """Packed cold/hot key lists (within-seq indices) + kind flags.

    kind: 0 = vertical (keep iff residue matches head), 1 = window (keep iff
    causal), 2 = dead filler.
    """
    res = (4, 5, 6, 7) if core % 2 == 0 else (0, 1, 2, 3)
    ck, cf, hk, hf = [], [], [], []

    def blk(keys, flags, b, kd):
        keys.extend(range(b * 64, b * 64 + 64))
        flags.extend([kd] * 64)

    for grp in range(g["nfull"]):
        for r in res:
            blk(ck, cf, grp * 8 + r, 0)
    have = [r for r in res if r < g["rp"]]
    for i in range(g["npart"]):
        if i < len(have):
            blk(ck, cf, g["gp"] * 8 + have[i], 0)
        else:
            blk(ck, cf, g["b0"], 2)
    hot0 = g["qb"] - g["hot"] + 1
    for b in range(g["b0"], hot0):
        blk(ck, cf, b, 1)
    for b in range(hot0, g["qb"] + 1):
        blk(hk, hf, b, 1)

    def pad(keys, flags, n):
        while len(keys) < n:
            keys.append(g["b0"] * 64)
            flags.append(2)
        return (np.asarray(keys[:n], dtype=np.int64),
                np.asarray(flags[:n], dtype=np.int64))

    ck, cf = pad(ck, cf, 128 * g["nch8"])
    hk, hf = pad(hk, hf, 128 * g["nchh"])
    return ck, cf, hk, hf


def _bias_for(core, keys, kind, qpos):
    """[n, 4] additive mask bias in packed order."""
    kb = keys // SPARSE_BS
    h = core * R + np.arange(R)
    vert_keep = (kb[:, None] + h[None, :] + 1) % VERT_STRIDE == 0
    win_keep = (keys <= qpos)[:, None]
    keep = np.where(kind[:, None] == 0, vert_keep,
                    np.where(kind[:, None] == 1, win_keep, False))
    return np.where(keep, np.float32(0.0), np.float32(NEG))


def _layout(context_lens):
    cl = np.asarray(context_lens)
    geos = [_geom(int(cl[s])) for s in range(NUM_SEQS)]
    order = sorted(range(NUM_SEQS), key=lambda s: -geos[s]["nch"])
    return cl, geos, order


def _pack_v(vsel, nch):
    """[(128*nch), 128] -> [128, nch*129] with a ones column per chunk."""
    v3 = vsel.reshape(nch, 128, HEAD_SIZE).transpose(1, 0, 2)
    out = np.empty((HEAD_SIZE, nch, 129), np.float32)
    out[:, :, :128] = v3
    out[:, :, 128] = 1.0
    return out.reshape(HEAD_SIZE, nch * 129)


def _build_host_arrays(q, k_cache, v_cache, block_tables, context_lens):
    cl, geos, order = _layout(context_lens)
    bt = np.asarray(block_tables)
    n8 = sum(g["nch8"] for g in geos)
    nh = sum(g["nchh"] for g in geos)
    nc_tot = n8 + nh

    q = np.asarray(q, np.float32)
    in_maps = []
    for c in range(N_KV_HEADS):
        k8 = np.empty((HEAD_SIZE, 128 * n8), E3M4)
        v8 = np.empty((HEAD_SIZE, 129 * n8), E3M4)
        kH = np.empty((HEAD_SIZE, 128 * nh + 4 * nc_tot), BF16)
        vH = np.empty((HEAD_SIZE, 129 * nh), BF16)
        o8 = oh = och = 0
        for s in order:
            g = geos[s]
            ck, cf, hk, hf = _keys_for(c, g)
            ks = k_cache[bt[s], c].transpose(1, 0, 2).reshape(HEAD_SIZE, MAX_SEQLEN)
            vs = v_cache[bt[s], c].transpose(0, 2, 1).reshape(MAX_SEQLEN, HEAD_SIZE)
            n8s, nhs, nchs = g["nch8"], g["nchh"], g["nch"]
            k8[:, 128 * o8: 128 * (o8 + n8s)] = ks[:, ck].astype(E3M4)
            v8[:, 129 * o8: 129 * (o8 + n8s)] = _pack_v(vs[ck], n8s).astype(E3M4)
            kH[:, 128 * oh: 128 * (oh + nhs)] = ks[:, hk].astype(BF16)
            vH[:, 129 * oh: 129 * (oh + nhs)] = _pack_v(vs[hk], nhs).astype(BF16)
            bias = np.concatenate(
                [_bias_for(c, ck, cf, g["qpos"]),
                 _bias_for(c, hk, hf, g["qpos"])], axis=0)       # [128*nch, 4]
            kH[:, 128 * nh + 4 * och: 128 * nh + 4 * (och + nchs)] = (
                bias.reshape(nchs, 128, R).transpose(1, 0, 2)
                .reshape(128, R * nchs)).astype(BF16)
            oh += nhs
            o8 += n8s
            och += nchs
        qT = np.ascontiguousarray(
            q[:, c * R:(c + 1) * R, :].transpose(2, 0, 1).reshape(
                HEAD_SIZE, NUM_SEQS * R)).astype(BF16)
        in_maps.append({"k8": k8, "v8": v8, "kH": kH, "vH": vH, "qT": qT})
    return in_maps, geos, order, n8, nh


def _emulate_core(im, geos, order, n8, nh):
    """Numpy mirror of the device program."""
    k8, v8, kH, vH, qT = (np.asarray(im[k], np.float32)
                          for k in ("k8", "v8", "kH", "vH", "qT"))
    out = np.zeros((NUM_SEQS, R, HEAD_SIZE), np.float32)
    o8 = oh = och = 0
    for s in order:
        g = geos[s]
        n8s, nhs, nchs = g["nch8"], g["nchh"], g["nch"]
        kt = np.concatenate(
            [k8[:, 128 * o8: 128 * (o8 + n8s)],
             kH[:, 128 * oh: 128 * (oh + nhs)]], axis=1)
        bias = kH[:, 128 * nh + 4 * och: 128 * nh + 4 * (och + nchs)]
        bias = bias.reshape(128, nchs, R).transpose(1, 0, 2).reshape(-1, R)
        scores = kt.T @ qT[:, s * R:(s + 1) * R] + bias
        p = np.exp(SM_SCALE * scores)
        acc = np.zeros((R, 129), np.float32)
        for i in range(n8s):
            acc += p[128 * i: 128 * (i + 1)].T @ v8[:, 129 * (o8 + i): 129 * (o8 + i + 1)]
        for i in range(nhs):
            acc += (p[128 * (n8s + i): 128 * (n8s + i + 1)].T
                    @ vH[:, 129 * (oh + i): 129 * (oh + i + 1)])
        out[s] = acc[:, :128] / acc[:, 128:129]
        o8 += n8s
        oh += nhs
        och += nchs
    return out


def _build_program(geos, order, n8, nh, kv_bufs=3):
    import concourse.bacc as bacc
    import concourse.tile as tile
    from concourse import mybir

    f32 = mybir.dt.float32
    bf16 = mybir.dt.bfloat16
    e3 = mybir.dt.float8e3
    nc = bacc.Bacc("TRN2", target_bir_lowering=False, debug=False, num_devices=8)
    nc_tot = n8 + nh

    k8D = nc.dram_tensor("k8", [HEAD_SIZE, 128 * n8], e3, kind="ExternalInput")
    v8D = nc.dram_tensor("v8", [HEAD_SIZE, 129 * n8], e3, kind="ExternalInput")
    kHD = nc.dram_tensor("kH", [HEAD_SIZE, 128 * nh + 4 * nc_tot], bf16,
                         kind="ExternalInput")
    vHD = nc.dram_tensor("vH", [HEAD_SIZE, 129 * nh], bf16, kind="ExternalInput")
    qTD = nc.dram_tensor("qT", [HEAD_SIZE, NUM_SEQS * R], bf16, kind="ExternalInput")
    outD = nc.dram_tensor("out", [R, NUM_SEQS * HEAD_SIZE], f32,
                          kind="ExternalOutput")

    # group boundaries (in sorted-seq space)
    groups = []
    i = 0
    for n in GROUPS:
        groups.append(order[i:i + n])
        i += n
    NCHMAX = max(g["nch"] for g in geos)
    gsz8 = [sum(geos[s]["nch8"] for s in grp) for grp in groups]
    gszh = [sum(geos[s]["nchh"] for s in grp) for grp in groups]
    G8MAX, GHMAX = max(gsz8), max(gszh)

    with tile.TileContext(nc) as tc:
        with (
            tc.tile_pool(name="const", bufs=1) as constp,
            tc.tile_pool(name="k8p", bufs=kv_bufs) as k8p,
            tc.tile_pool(name="v8p", bufs=kv_bufs) as v8p,
            tc.tile_pool(name="khp", bufs=kv_bufs) as khp,
            tc.tile_pool(name="vhp", bufs=kv_bufs) as vhp,
            tc.tile_pool(name="p", bufs=4) as pp,
            tc.tile_pool(name="o", bufs=4) as op,
            tc.tile_pool(name="ps_s", bufs=4, space="PSUM") as ps_s,
            tc.tile_pool(name="ps_o", bufs=4, space="PSUM") as ps_o,
        ):
            qt = constp.tile([HEAD_SIZE, NUM_SEQS * R], bf16)
            nc.sync.dma_start(qt[:], qTD[:])
            bt_ = constp.tile([HEAD_SIZE, 4 * nc_tot], bf16)
            nc.sync.dma_start(bt_[:], kHD[:, 128 * nh:])
            outacc = constp.tile([R, NUM_SEQS * HEAD_SIZE], f32)

            o8 = oh = och = 0
            for gi, grp in enumerate(groups):
                c8, ch = gsz8[gi], gszh[gi]
                k8t = k8p.tile([HEAD_SIZE, 128 * G8MAX], e3, tag="k8")
                v8t = v8p.tile([HEAD_SIZE, 129 * G8MAX], e3, tag="v8")
                kht = khp.tile([HEAD_SIZE, 128 * GHMAX], bf16, tag="kh")
                vht = vhp.tile([HEAD_SIZE, 129 * GHMAX], bf16, tag="vh")
                if c8:
                    nc.sync.dma_start(k8t[:, :128 * c8],
                                      k8D[:, 128 * o8: 128 * (o8 + c8)])
                    nc.scalar.dma_start(v8t[:, :129 * c8],
                                        v8D[:, 129 * o8: 129 * (o8 + c8)])
                nc.sync.dma_start(kht[:, :128 * ch],
                                  kHD[:, 128 * oh: 128 * (oh + ch)])
                nc.scalar.dma_start(vht[:, :129 * ch],
                                    vHD[:, 129 * oh: 129 * (oh + ch)])

                b8 = bh = 0   # offsets inside the group tiles
                for s in grp:
                    g = geos[s]
                    n8s, nhs, nchs = g["nch8"], g["nchh"], g["nch"]
                    sc_ps = ps_s.tile([128, R * NCHMAX], f32, tag="sc")
                    for i in range(n8s):
                        nc.tensor.matmul(
                            sc_ps[:, R * i: R * (i + 1)],
                            k8t[:, 128 * (b8 + i): 128 * (b8 + i + 1)],
                            qt[:, s * R:(s + 1) * R], start=True, stop=True)
                    for i in range(nhs):
                        nc.tensor.matmul(
                            sc_ps[:, R * (n8s + i): R * (n8s + i + 1)],
                            kht[:, 128 * (bh + i): 128 * (bh + i + 1)],
                            qt[:, s * R:(s + 1) * R], start=True, stop=True)
                    nc.vector.tensor_add(
                        sc_ps[:, : R * nchs], sc_ps[:, : R * nchs],
                        bt_[:, R * och: R * (och + nchs)])
                    p_all = pp.tile([128, R * NCHMAX], bf16, tag="pall")
                    nc.scalar.activation(
                        p_all[:, : R * nchs], sc_ps[:, : R * nchs],
                        mybir.ActivationFunctionType.Exp, scale=float(SM_SCALE))

                    out_ps = ps_o.tile([R, 129], f32, tag="ops")
                    for i in range(n8s):
                        nc.tensor.matmul(
                            out_ps[:], p_all[:, R * i: R * (i + 1)],
                            v8t[:, 129 * (b8 + i): 129 * (b8 + i + 1)],
                            start=(i == 0), stop=False)
                    for i in range(nhs):
                        nc.tensor.matmul(
                            out_ps[:], p_all[:, R * (n8s + i): R * (n8s + i + 1)],
                            vht[:, 129 * (bh + i): 129 * (bh + i + 1)],
                            start=(n8s + i == 0), stop=(i == nhs - 1))
                    rsum = op.tile([R, 1], f32, tag="rsum")
                    nc.vector.reciprocal(rsum[:], out_ps[:, 128:129])
                    nc.vector.tensor_scalar_mul(
                        outacc[:, HEAD_SIZE * s: HEAD_SIZE * (s + 1)],
                        out_ps[:, :128], rsum[:])
                    b8 += n8s
                    bh += nhs
                    o8 += n8s
                    oh += nhs
                    och += nchs
            nc.sync.dma_start(outD[:], outacc[:])
    nc.finalize()
    return nc


def kernel(q, k_cache, v_cache, block_tables, context_lens, _emulate=False):
    in_maps, geos, order, n8, nh = _build_host_arrays(
        q, k_cache, v_cache, block_tables, context_lens)

    if _emulate:
        outs = [_emulate_core(in_maps[c], geos, order, n8, nh)
                for c in range(N_KV_HEADS)]
    else:
        import os
        from concourse.bass_utils import run_bass_kernel_spmd
        nc = _build_program(geos, order, n8, nh)
        kw = {}
        if os.environ.get("KERNEL_TRACE"):
            kw = dict(trace=True, trace_cores=list(range(8)),
                      tmpdir=os.environ.get("KERNEL_TRACE_DIR") or None)
        br = run_bass_kernel_spmd(nc, in_maps, list(range(8)), **kw)
        global LAST_EXEC_NS, LAST_RESULTS
        LAST_RESULTS = br
        LAST_EXEC_NS = br.exec_time_ns
        outs = [np.asarray(br.results[c]["out"]).reshape(
            R, NUM_SEQS, HEAD_SIZE).transpose(1, 0, 2)
            for c in range(N_KV_HEADS)]

    out = np.zeros((NUM_SEQS, N_Q_HEADS, HEAD_SIZE), np.float32)
    for c in range(N_KV_HEADS):
        out[:, c * R:(c + 1) * R, :] = outs[c]
    return out
